# revision 17
# baseline (speedup 1.0000x reference)
"""Trainium2 Bass kernel for the MAB (multihead attention block) problem.

Full inputs in, full outputs out. Data-parallel over batch: 16 batches
across 8 NeuronCores = 2 batches/core. No collectives.

Per-core pipeline (per batch):
  1. QpT = (Q @ Wq + bq)^T, KpT likewise, Vp natural (+ interleaved ones
     column per head for the softmax denominator).
  2. Per head pair: S^T = Kh @ Qh^T (row-tiled K=64 pairs sharing the PE
     array), P = exp(S^T*s) in one [128,1024] activation (no max
     subtraction -- scores are N(0, 0.35), exp is safe),
     O'^T[65, nq] = [Vh | 1]^T @ P accumulated over nk chunks
     (row 64 = softmax denominator). PV matmuls are software-pipelined
     one nk-tile behind the score matmuls so the PE never waits on exp.
  3. Transpose O'^T to natural, normalize rows by 1/denominator, add Qp
     residual.
  4. LayerNorm -> transpose -> FFN (relu(X @ Wo + bo)) + residual ->
     LayerNorm -> out. Affine params that are identically (1, 0) are
     folded out at build time (checked against the actual input values).
"""

import math
import sys
from contextlib import ExitStack

import numpy as np

sys.path.insert(0, "/opt/trn_rl_repo")

import concourse.bass as bass
import concourse.tile as tile
from concourse import bacc
from concourse import mybir
from concourse.bass import ds, ts
from concourse.bass_utils import run_bass_kernel_spmd
from concourse.masks import make_identity

FP = mybir.dt.float32
AF = mybir.ActivationFunctionType
ALU = mybir.AluOpType
FR = mybir.dt.float32r

B, N, D = 16, 1024, 512
NCORES = 8
BL = B // NCORES  # batches per core
H, HD = 8, 64
PAIRS = H // 2
SCALE = 1.0 / math.sqrt(D)
EPS = 1e-5
P = 128
DT = D // P  # 4 dv chunks
NT = N // P  # 8 nq/nk tiles
HA = HD + 1  # head dim + denominator column


def _bcast_ap(ap):
    """Broadcast a [D]-shaped DRAM AP across all 128 partitions."""
    return bass.AP(tensor=ap.tensor, offset=ap.offset, ap=[[0, P]] + list(ap.ap))


def _build_program(triv0, triv1, trivbo):
    nc = bacc.Bacc(None, target_bir_lowering=False)
    dr = {}
    for name, shape in [
        ("QT", [BL, D, N]),
        ("KT", [BL, D, N]),
        ("Wq", [D, D]),
        ("Wk", [D, D]),
        ("Wv", [D, D]),
        ("Wo", [D, D]),
        ("bq2", [P, DT]),
        ("bk2", [P, DT]),
        ("bv", [D]),
        ("bo", [D]),
        ("g0", [D]),
        ("b0", [D]),
        ("g1", [D]),
        ("b1", [D]),
    ]:
        dt = FR if name in ("QT", "KT", "Wq", "Wk", "Wv", "Wo") else FP
        dr[name] = nc.declare_dram_parameter(name, shape, dt, isOutput=False)
    out_O = nc.declare_dram_parameter("O", [BL, N, D], FP, isOutput=True)

    qt_src = dr["QT"][:].rearrange("b (c p) n -> b p c n", p=P)
    kt_src = dr["KT"][:].rearrange("b (c p) n -> b p c n", p=P)

    with tile.TileContext(nc) as tc, ExitStack() as ctx:
        singles = ctx.enter_context(tc.tile_pool(name="singles", bufs=1))
        work = ctx.enter_context(tc.tile_pool(name="work", bufs=1))
        pch = ctx.enter_context(tc.tile_pool(name="pch", bufs=3))
        lnt = ctx.enter_context(tc.tile_pool(name="lnt", bufs=2))
        ost = ctx.enter_context(tc.tile_pool(name="ost", bufs=1))
        otile = ctx.enter_context(tc.tile_pool(name="otile", bufs=2))
        sml = ctx.enter_context(tc.tile_pool(name="sml", bufs=8))
        # PSUM budget: acc 2 banks + opv 2 banks + flow 2x2 banks = 8
        ps_acc = ctx.enter_context(tc.tile_pool(name="ps_acc", bufs=2, space="PSUM"))
        ps_pv = ctx.enter_context(tc.tile_pool(name="ps_pv", bufs=1, space="PSUM"))
        ps_flow = ctx.enter_context(tc.tile_pool(name="ps_flow", bufs=2, space="PSUM"))

        # ---- statics
        wsb = {}
        for wname in ("Wq", "Wk", "Wv", "Wo"):
            w = singles.tile([P, DT, D], FR, tag=wname)
            nc.sync.dma_start(out=w, in_=dr[wname][:].rearrange("(c p) d -> p c d", p=P))
            wsb[wname] = w
        bq_sb = singles.tile([P, DT], FP, tag="bq2")
        nc.sync.dma_start(out=bq_sb, in_=dr["bq2"][:])
        bk_sb = singles.tile([P, DT], FP, tag="bk2")
        nc.sync.dma_start(out=bk_sb, in_=dr["bk2"][:])
        bc = {}
        for bname in ("bv", "bo", "g0", "b0", "g1", "b1"):
            t = singles.tile([P, D], FP, tag=bname)
            nc.gpsimd.dma_start(out=t, in_=_bcast_ap(dr[bname][:]))
            bc[bname] = t
        ident = singles.tile([P, P], FP, tag="ident")
        make_identity(nc, ident)
        ident_r = singles.tile([P, P], FR, tag="identr")
        nc.vector.tensor_copy(ident_r, ident)
        eps_sb = singles.tile([P, 1], FP, tag="eps")
        nc.vector.memset(eps_sb, EPS)
        ones8 = singles.tile([P, H, 1], FP, tag="ones8")
        nc.vector.memset(ones8, 1.0)

        for b in range(BL):
            # ---------- phase A: projections ----------
            qt = work.tile([P, DT, N], FR, tag="qt")
            kt = work.tile([P, DT, N], FR, tag="kt")
            for c in range(DT):
                nc.sync.dma_start(out=qt[:, c, :], in_=qt_src[b, :, c, :])
                nc.sync.dma_start(out=kt[:, c, :], in_=kt_src[b, :, c, :])

            qpt = work.tile([P, DT, N], FR, tag="qpt")
            kpt = work.tile([P, DT, N], FR, tag="kpt")
            for dst, w, bias, src in (
                (qpt, wsb["Wq"], bq_sb, qt),
                (kpt, wsb["Wk"], bk_sb, kt),
            ):
                for t in range(DT):
                    ps = ps_flow.tile([P, N], FP, tag="flow", name="projps")
                    for hf in range(2):
                        for c in range(DT):
                            nc.tensor.matmul(
                                ps[:, ds(hf * 512, 512)],
                                (w[:, c, ts(t, P)]),
                                (src[:, c, ds(hf * 512, 512)]),
                                start=(c == 0),
                                stop=(c == DT - 1),
                            )
                    # PSUM -> SBUF with per-partition bias add
                    nc.vector.tensor_scalar_add(dst[:, t, :], ps, bias[:, t : t + 1])

            # Vp natural, augmented: per head 64 V columns + a ones column
            vpa = work.tile([P, NT, H * HA], FR, tag="vpa")
            for m in range(NT):
                ps = ps_acc.tile([P, 512], FP, tag="acc", name="vps")
                for c in range(DT):
                    nc.tensor.matmul(
                        ps,
                        (kt[:, c, ts(m, P)]),
                        (wsb["Wv"][:, c, :]),
                        start=(c == 0),
                        stop=(c == DT - 1),
                    )
                vslice = vpa[:, m, :].rearrange("p (h s) -> p h s", s=HA)
                nc.vector.scalar_tensor_tensor(
                    out=vslice[:, :, 0:HD],
                    in0=ps[:, :].rearrange("p (h s) -> p h s", s=HD),
                    scalar=0.0,
                    in1=bc["bv"][:, :].rearrange("p (h s) -> p h s", s=HD),
                    op0=ALU.bypass,
                    op1=ALU.add,
                )
                nc.vector.tensor_copy(vslice[:, :, HD : HD + 1], ones8)

            # Qp natural (for the attention residual): transpose QpT blocks
            qp = work.tile([P, NT, D], FP, tag="qp")
            for m in range(NT):
                tp = ps_acc.tile([P, 512], FR, tag="acc", name="qptr")
                for t in range(DT):
                    nc.tensor.transpose(tp[:, ts(t, P)], qpt[:, t, ts(m, P)], ident_r)
                nc.any.tensor_copy(qp[:, m, :], tp)

            # ---------- phase B: attention ----------
            oasm = work.tile([P, NT, D], FP, tag="qt2")
            ln1 = work.tile([P, NT, D], FP, tag="kt2")

            def emit_ln1(q):
                x = oasm[:, q, :]
                st = sml.tile([P, 6], FP, tag="bn", name="st")
                nc.vector.bn_stats(st, x)
                mv = sml.tile([P, 2], FP, tag="mv", name="mv")
                nc.vector.bn_aggr(mv, st)
                rs = sml.tile([P, 1], FP, tag="rs", name="rs")
                nc.scalar.activation(rs, mv[:, 1:2], AF.Sqrt, bias=eps_sb)
                nc.vector.reciprocal(rs, rs)
                lq = ln1[:, q, :]
                nc.vector.tensor_scalar(
                    out=lq,
                    in0=x,
                    scalar1=mv[:, 0:1],
                    scalar2=rs,
                    op0=ALU.subtract,
                    op1=ALU.mult,
                )
                if not triv0:
                    nc.vector.tensor_tensor(lq, lq, bc["g0"], ALU.mult)
                    nc.vector.tensor_tensor(lq, lq, bc["b0"], ALU.add)

            for hf in range(2):
                for hp in range(PAIRS):
                    qslice = ds(hf * 512, 512)
                    o_pair = ps_pv.tile([HA, N], FP, tag="opv")
                    prev = None
                    for m in range(NT):
                        s_pair = ps_flow.tile([P, N], FP, tag="flow", name="spair")
                        for j in range(2):
                            lo = j * 64
                            nc.tensor.matmul(
                                s_pair[:, ds(j * 512, 512)],
                                (kpt[lo : lo + 64, hp, ts(m, P)]),
                                (qpt[lo : lo + 64, hp, qslice]),
                                start=True,
                                stop=True,
                            )
                        p_pair = pch.tile([P, N], FR, tag="p")
                        nc.scalar.activation(p_pair, s_pair, AF.Exp, scale=SCALE)
                        if prev is not None:
                            pm, pp = prev
                            for j in range(2):
                                nc.tensor.matmul(
                                    o_pair[:, ds(j * 512, 512)],
                                    (vpa[:, pm, :].rearrange("p (h s) -> p h s", s=HA)[
                                        :, 2 * hp + j, :
                                    ]),
                                    (pp[:, ds(j * 512, 512)]),
                                    start=(pm == 0),
                                    stop=(pm == NT - 1),
                                )
                        prev = (m, p_pair)
                    pm, pp = prev
                    for j in range(2):
                        nc.tensor.matmul(
                            o_pair[:, ds(j * 512, 512)],
                            (vpa[:, pm, :].rearrange("p (h s) -> p h s", s=HA)[
                                :, 2 * hp + j, :
                            ]),
                            (pp[:, ds(j * 512, 512)]),
                            start=(pm == 0),
                            stop=(pm == NT - 1),
                        )
                    # drain: PSUM -> SBUF, transpose to natural, normalize, add Qp
                    o_sb = ost.tile([HA, N], FP, tag="ost")
                    nc.any.tensor_copy(o_sb[:, 0:512], o_pair[:, 0:512])
                    nc.any.tensor_copy(o_sb[:, 512:1024], o_pair[:, 512:1024])
                    for j in range(2):
                        h = 2 * hp + j
                        t_ps = ps_acc.tile([P, 4 * HA], FP, tag="acc", name="otr")
                        for qq in range(4):
                            nc.tensor.transpose(
                                t_ps[:, ds(qq * HA, HA)],
                                o_sb[:, ds(j * 512 + qq * P, P)],
                                ident[0:HA, 0:HA],
                            )
                        for qq in range(4):
                            q = hf * 4 + qq
                            r = sml.tile([P, 1], FP, tag="r")
                            nc.vector.reciprocal(r, t_ps[:, ds(qq * HA + HD, 1)])
                            nc.vector.scalar_tensor_tensor(
                                out=oasm[:, q, ds(h * HD, HD)],
                                in0=t_ps[:, ds(qq * HA, HD)],
                                scalar=r,
                                in1=qp[:, q, ds(h * HD, HD)],
                                op0=ALU.mult,
                                op1=ALU.add,
                            )

            # ---------- phase C: LN1 passes then FFN + LN2 ----------
            for q in range(NT):
                emit_ln1(q)

            rlf = work.tile([P, NT, D], FP, tag="qt")
            for q in range(NT):
                lq = ln1[:, q, :]
                tp = ps_flow.tile([P, N], FP, tag="flow", name="lntr")
                for c in range(DT):
                    nc.tensor.transpose(tp[:, ts(c, P)], lq[:, ts(c, P)], ident)
                l_t = lnt.tile([P, DT, P], FR, tag="lnt")
                nc.any.tensor_copy(l_t, tp[:, 0:512].rearrange("p (c n) -> p c n", n=P))

                f_ps = ps_acc.tile([P, 512], FP, tag="acc", name="ffps")
                for c in range(DT):
                    nc.tensor.matmul(
                        f_ps,
                        (l_t[:, c, :]),
                        (wsb["Wo"][:, c, :]),
                        start=(c == 0),
                        stop=(c == DT - 1),
                    )
                if trivbo:
                    nc.vector.tensor_scalar_max(rlf[:, q, :], f_ps, 0.0)
                else:
                    nc.vector.scalar_tensor_tensor(
                        out=rlf[:, q, :],
                        in0=f_ps,
                        scalar=0.0,
                        in1=bc["bo"],
                        op0=ALU.bypass,
                        op1=ALU.add,
                    )
                    nc.vector.tensor_scalar_max(rlf[:, q, :], rlf[:, q, :], 0.0)

            for q in range(NT):
                o2 = otile.tile([P, D], FP, tag="o2")
                nc.vector.tensor_tensor(o2, ln1[:, q, :], rlf[:, q, :], ALU.add)
                st2 = sml.tile([P, 6], FP, tag="bn")
                nc.vector.bn_stats(st2, o2)
                mv2 = sml.tile([P, 2], FP, tag="mv")
                nc.vector.bn_aggr(mv2, st2)
                rs2 = sml.tile([P, 1], FP, tag="rs")
                nc.scalar.activation(rs2, mv2[:, 1:2], AF.Sqrt, bias=eps_sb)
                nc.vector.reciprocal(rs2, rs2)
                z2 = otile.tile([P, D], FP, tag="z")
                nc.vector.tensor_scalar(
                    out=z2,
                    in0=o2,
                    scalar1=mv2[:, 0:1],
                    scalar2=rs2,
                    op0=ALU.subtract,
                    op1=ALU.mult,
                )
                if not triv1:
                    nc.vector.tensor_tensor(z2, z2, bc["g1"], ALU.mult)
                    nc.vector.tensor_tensor(z2, z2, bc["b1"], ALU.add)
                nc.sync.dma_start(out=out_O[b, ts(q, P), :], in_=z2)

    nc.compile()
    return nc


_NC = {}


def _get_nc(triv0, triv1, trivbo):
    key = (triv0, triv1, trivbo)
    if key not in _NC:
        _NC[key] = _build_program(*key)
    return _NC[key]


def _prep_in_maps(inputs):
    f32 = lambda x: np.ascontiguousarray(np.asarray(x), dtype=np.float32)
    Q, K = f32(inputs["Q"]), f32(inputs["K"])
    QT = np.ascontiguousarray(Q.transpose(0, 2, 1))
    KT = np.ascontiguousarray(K.transpose(0, 2, 1))
    shared = {
        "Wq": f32(inputs["Wq"]),
        "Wk": f32(inputs["Wk"]),
        "Wv": f32(inputs["Wv"]),
        "Wo": f32(inputs["Wo"]),
        "bq2": np.ascontiguousarray(f32(inputs["bq"]).reshape(DT, P).T),
        "bk2": np.ascontiguousarray(f32(inputs["bk"]).reshape(DT, P).T),
        "bv": f32(inputs["bv"]),
        "bo": f32(inputs["bo"]),
        "g0": f32(inputs["g0"]),
        "b0": f32(inputs["b0"]),
        "g1": f32(inputs["g1"]),
        "b1": f32(inputs["b1"]),
    }
    in_maps = []
    for c in range(NCORES):
        m = dict(shared)
        m["QT"] = np.ascontiguousarray(QT[c * BL : (c + 1) * BL])
        m["KT"] = np.ascontiguousarray(KT[c * BL : (c + 1) * BL])
        in_maps.append(m)
    return in_maps


def _run(inputs, trace=False):
    triv0 = bool(
        np.all(np.asarray(inputs["g0"]) == 1.0)
        and np.all(np.asarray(inputs["b0"]) == 0.0)
    )
    triv1 = bool(
        np.all(np.asarray(inputs["g1"]) == 1.0)
        and np.all(np.asarray(inputs["b1"]) == 0.0)
    )
    trivbo = bool(np.all(np.asarray(inputs["bo"]) == 0.0))
    nc = _get_nc(triv0, triv1, trivbo)
    in_maps = _prep_in_maps(inputs)
    return run_bass_kernel_spmd(nc, in_maps, list(range(NCORES)), trace=trace)


def kernel(**inputs):
    res = _run(inputs, trace=False)
    return np.concatenate([res.results[c]["O"] for c in range(NCORES)], axis=0)


# revision 18
# speedup vs baseline: 1.0220x; 1.0220x over previous
"""Trainium2 Bass kernel for the MAB (multihead attention block) problem.

Full inputs in, full outputs out. Data-parallel over batch: 16 batches
across 8 NeuronCores = 2 batches/core. No collectives.

Per-core pipeline (per batch):
  1. QpT = (Q @ Wq + bq)^T, KpT likewise, Vp natural (+ interleaved ones
     column per head for the softmax denominator).
  2. Per head pair: S^T = Kh @ Qh^T (row-tiled K=64 pairs sharing the PE
     array), P = exp(S^T*s) in one [128,1024] activation (no max
     subtraction -- scores are N(0, 0.35), exp is safe),
     O'^T[65, nq] = [Vh | 1]^T @ P accumulated over nk chunks
     (row 64 = softmax denominator). PV matmuls are software-pipelined
     one nk-tile behind the score matmuls so the PE never waits on exp.
  3. Transpose O'^T to natural, normalize rows by 1/denominator, add Qp
     residual.
  4. LayerNorm -> transpose -> FFN (relu(X @ Wo + bo)) + residual ->
     LayerNorm -> out. Affine params that are identically (1, 0) are
     folded out at build time (checked against the actual input values).
"""

import math
import sys
from contextlib import ExitStack

import numpy as np

sys.path.insert(0, "/opt/trn_rl_repo")

import concourse.bass as bass
import concourse.tile as tile
from concourse import bacc
from concourse import mybir
from concourse.bass import ds, ts
from concourse.bass_utils import run_bass_kernel_spmd
from concourse.masks import make_identity

FP = mybir.dt.float32
AF = mybir.ActivationFunctionType
ALU = mybir.AluOpType
FR = mybir.dt.float32r

B, N, D = 16, 1024, 512
NCORES = 8
BL = B // NCORES  # batches per core
H, HD = 8, 64
PAIRS = H // 2
SCALE = 1.0 / math.sqrt(D)
EPS = 1e-5
P = 128
DT = D // P  # 4 dv chunks
NT = N // P  # 8 nq/nk tiles
HA = HD + 1  # head dim + denominator column


def _bcast_ap(ap):
    """Broadcast a [D]-shaped DRAM AP across all 128 partitions."""
    return bass.AP(tensor=ap.tensor, offset=ap.offset, ap=[[0, P]] + list(ap.ap))


def _build_program(triv0, triv1, trivbo):
    nc = bacc.Bacc(None, target_bir_lowering=False)
    dr = {}
    for name, shape in [
        ("QT", [BL, D, N]),
        ("KT", [BL, D, N]),
        ("Wq", [D, D]),
        ("Wk", [D, D]),
        ("Wv", [D, D]),
        ("Wo", [D, D]),
        ("bq2", [P, DT]),
        ("bk2", [P, DT]),
        ("bv", [D]),
        ("bo", [D]),
        ("g0", [D]),
        ("b0", [D]),
        ("g1", [D]),
        ("b1", [D]),
    ]:
        dt = FR if name in ("QT", "KT", "Wq", "Wk", "Wv", "Wo") else FP
        dr[name] = nc.declare_dram_parameter(name, shape, dt, isOutput=False)
    out_O = nc.declare_dram_parameter("O", [BL, N, D], FP, isOutput=True)

    qt_src = dr["QT"][:].rearrange("b (c p) n -> b p c n", p=P)
    kt_src = dr["KT"][:].rearrange("b (c p) n -> b p c n", p=P)

    with tile.TileContext(nc) as tc, ExitStack() as ctx:
        singles = ctx.enter_context(tc.tile_pool(name="singles", bufs=1))
        work = ctx.enter_context(tc.tile_pool(name="work", bufs=1))
        pch = ctx.enter_context(tc.tile_pool(name="pch", bufs=3))
        lnt = ctx.enter_context(tc.tile_pool(name="lnt", bufs=2))
        ost = ctx.enter_context(tc.tile_pool(name="ost", bufs=1))
        otile = ctx.enter_context(tc.tile_pool(name="otile", bufs=2))
        sml = ctx.enter_context(tc.tile_pool(name="sml", bufs=8))
        # PSUM budget: acc 2 banks + opv 2 banks + flow 2x2 banks = 8
        ps_acc = ctx.enter_context(tc.tile_pool(name="ps_acc", bufs=2, space="PSUM"))
        ps_pv = ctx.enter_context(tc.tile_pool(name="ps_pv", bufs=1, space="PSUM"))
        ps_flow = ctx.enter_context(tc.tile_pool(name="ps_flow", bufs=2, space="PSUM"))

        # ---- statics
        wsb = {}
        for wname in ("Wq", "Wk", "Wv", "Wo"):
            w = singles.tile([P, DT, D], FR, tag=wname)
            nc.sync.dma_start(out=w, in_=dr[wname][:].rearrange("(c p) d -> p c d", p=P))
            wsb[wname] = w
        bq_sb = singles.tile([P, DT], FP, tag="bq2")
        nc.sync.dma_start(out=bq_sb, in_=dr["bq2"][:])
        bk_sb = singles.tile([P, DT], FP, tag="bk2")
        nc.sync.dma_start(out=bk_sb, in_=dr["bk2"][:])
        bc = {}
        for bname in ("bv", "bo", "g0", "b0", "g1", "b1"):
            t = singles.tile([P, D], FP, tag=bname)
            nc.gpsimd.dma_start(out=t, in_=_bcast_ap(dr[bname][:]))
            bc[bname] = t
        ident = singles.tile([P, P], FP, tag="ident")
        make_identity(nc, ident)
        ident_r = singles.tile([P, P], FR, tag="identr")
        nc.vector.tensor_copy(ident_r, ident)
        eps_sb = singles.tile([P, 1], FP, tag="eps")
        nc.vector.memset(eps_sb, EPS)
        ones8 = singles.tile([P, H, 1], FP, tag="ones8")
        nc.vector.memset(ones8, 1.0)

        for b in range(BL):
            # ---------- phase A: projections ----------
            qt = work.tile([P, DT, N], FR, tag="qt")
            kt = work.tile([P, DT, N], FR, tag="kt")
            for c in range(DT):
                nc.sync.dma_start(out=qt[:, c, :], in_=qt_src[b, :, c, :])
                nc.sync.dma_start(out=kt[:, c, :], in_=kt_src[b, :, c, :])

            qpt = work.tile([P, DT, N], FR, tag="qpt")
            kpt = work.tile([P, DT, N], FR, tag="kpt")
            for dst, w, bias, src in (
                (qpt, wsb["Wq"], bq_sb, qt),
                (kpt, wsb["Wk"], bk_sb, kt),
            ):
                for t in range(DT):
                    ps = ps_flow.tile([P, N], FP, tag="flow", name="projps")
                    for hf in range(2):
                        for c in range(DT):
                            nc.tensor.matmul(
                                ps[:, ds(hf * 512, 512)],
                                (w[:, c, ts(t, P)]),
                                (src[:, c, ds(hf * 512, 512)]),
                                start=(c == 0),
                                stop=(c == DT - 1),
                            )
                    # PSUM -> SBUF with per-partition bias add
                    nc.vector.tensor_scalar_add(dst[:, t, :], ps, bias[:, t : t + 1])

            # Vp natural, augmented: per head 64 V columns + a ones column
            vpa = work.tile([P, NT, H * HA], FR, tag="vpa")
            for m in range(NT):
                ps = ps_acc.tile([P, 512], FP, tag="acc", name="vps")
                for c in range(DT):
                    nc.tensor.matmul(
                        ps,
                        (kt[:, c, ts(m, P)]),
                        (wsb["Wv"][:, c, :]),
                        start=(c == 0),
                        stop=(c == DT - 1),
                    )
                vslice = vpa[:, m, :].rearrange("p (h s) -> p h s", s=HA)
                nc.vector.scalar_tensor_tensor(
                    out=vslice[:, :, 0:HD],
                    in0=ps[:, :].rearrange("p (h s) -> p h s", s=HD),
                    scalar=0.0,
                    in1=bc["bv"][:, :].rearrange("p (h s) -> p h s", s=HD),
                    op0=ALU.bypass,
                    op1=ALU.add,
                )
                nc.vector.tensor_copy(vslice[:, :, HD : HD + 1], ones8)

            # Qp natural (for the attention residual): transpose QpT blocks
            qp = work.tile([P, NT, D], FP, tag="qp")
            for m in range(NT):
                tp = ps_acc.tile([P, 512], FR, tag="acc", name="qptr")
                for t in range(DT):
                    nc.tensor.transpose(tp[:, ts(t, P)], qpt[:, t, ts(m, P)], ident_r)
                nc.any.tensor_copy(qp[:, m, :], tp)

            # ---------- phase B: attention ----------
            oasm = work.tile([P, NT, D], FP, tag="qt2")
            ln1 = work.tile([P, NT, D], FP, tag="kt2")

            def emit_ln1(q):
                x = oasm[:, q, :]
                st = sml.tile([P, 6], FP, tag="bn", name="st")
                nc.vector.bn_stats(st, x)
                mv = sml.tile([P, 2], FP, tag="mv", name="mv")
                nc.vector.bn_aggr(mv, st)
                rs = sml.tile([P, 1], FP, tag="rs", name="rs")
                nc.scalar.activation(rs, mv[:, 1:2], AF.Sqrt, bias=eps_sb)
                nc.vector.reciprocal(rs, rs)
                lq = ln1[:, q, :]
                nc.vector.tensor_scalar(
                    out=lq,
                    in0=x,
                    scalar1=mv[:, 0:1],
                    scalar2=rs,
                    op0=ALU.subtract,
                    op1=ALU.mult,
                )
                if not triv0:
                    nc.vector.tensor_tensor(lq, lq, bc["g0"], ALU.mult)
                    nc.vector.tensor_tensor(lq, lq, bc["b0"], ALU.add)

            for hf in range(2):
                for hp in range(PAIRS):
                    qslice = ds(hf * 512, 512)
                    o_pair = ps_pv.tile([HA, N], FP, tag="opv")
                    prev = None
                    for m in range(NT):
                        s_pair = ps_flow.tile([P, N], FP, tag="flow", name="spair")
                        for j in range(2):
                            lo = j * 64
                            nc.tensor.matmul(
                                s_pair[:, ds(j * 512, 512)],
                                (kpt[lo : lo + 64, hp, ts(m, P)]),
                                (qpt[lo : lo + 64, hp, qslice]),
                                start=True,
                                stop=True,
                            )
                        p_pair = pch.tile([P, N], FR, tag="p")
                        nc.scalar.activation(p_pair, s_pair, AF.Exp, scale=SCALE)
                        if prev is not None:
                            pm, pp = prev
                            for j in range(2):
                                nc.tensor.matmul(
                                    o_pair[:, ds(j * 512, 512)],
                                    (vpa[:, pm, :].rearrange("p (h s) -> p h s", s=HA)[
                                        :, 2 * hp + j, :
                                    ]),
                                    (pp[:, ds(j * 512, 512)]),
                                    start=(pm == 0),
                                    stop=(pm == NT - 1),
                                )
                        prev = (m, p_pair)
                    pm, pp = prev
                    for j in range(2):
                        nc.tensor.matmul(
                            o_pair[:, ds(j * 512, 512)],
                            (vpa[:, pm, :].rearrange("p (h s) -> p h s", s=HA)[
                                :, 2 * hp + j, :
                            ]),
                            (pp[:, ds(j * 512, 512)]),
                            start=(pm == 0),
                            stop=(pm == NT - 1),
                        )
                    # drain: PSUM -> SBUF, transpose to natural, normalize, add Qp
                    o_sb = ost.tile([HA, N], FP, tag="ost")
                    nc.any.tensor_copy(o_sb[:, 0:512], o_pair[:, 0:512])
                    nc.any.tensor_copy(o_sb[:, 512:1024], o_pair[:, 512:1024])
                    for j in range(2):
                        h = 2 * hp + j
                        t_ps = ps_acc.tile([P, 4 * HA], FP, tag="acc", name="otr")
                        for qq in range(4):
                            nc.tensor.transpose(
                                t_ps[:, ds(qq * HA, HA)],
                                o_sb[:, ds(j * 512 + qq * P, P)],
                                ident[0:HA, 0:HA],
                            )
                        for qq in range(4):
                            q = hf * 4 + qq
                            r = sml.tile([P, 1], FP, tag="r")
                            nc.vector.reciprocal(r, t_ps[:, ds(qq * HA + HD, 1)])
                            nc.vector.scalar_tensor_tensor(
                                out=oasm[:, q, ds(h * HD, HD)],
                                in0=t_ps[:, ds(qq * HA, HD)],
                                scalar=r,
                                in1=qp[:, q, ds(h * HD, HD)],
                                op0=ALU.mult,
                                op1=ALU.add,
                            )

            # ---------- phase C: LN1 passes then FFN + LN2 ----------
            for q in range(NT):
                emit_ln1(q)

            rlf = work.tile([P, NT, D], FP, tag="qt")
            for q in range(NT):
                lq = ln1[:, q, :]
                tp = ps_flow.tile([P, N], FP, tag="flow", name="lntr")
                for c in range(DT):
                    nc.tensor.transpose(tp[:, ts(c, P)], lq[:, ts(c, P)], ident)
                l_t = lnt.tile([P, DT, P], FR, tag="lnt")
                nc.any.tensor_copy(l_t, tp[:, 0:512].rearrange("p (c n) -> p c n", n=P))

                f_ps = ps_acc.tile([P, 512], FP, tag="acc", name="ffps")
                for c in range(DT):
                    nc.tensor.matmul(
                        f_ps,
                        (l_t[:, c, :]),
                        (wsb["Wo"][:, c, :]),
                        start=(c == 0),
                        stop=(c == DT - 1),
                    )
                if trivbo:
                    nc.scalar.activation(rlf[:, q, :], f_ps, AF.Relu)
                else:
                    nc.vector.tensor_tensor(rlf[:, q, :], f_ps, bc["bo"], ALU.add)
                    nc.scalar.activation(rlf[:, q, :], rlf[:, q, :], AF.Relu)

            for q in range(NT):
                o2 = otile.tile([P, D], FP, tag="o2")
                nc.vector.tensor_tensor(o2, ln1[:, q, :], rlf[:, q, :], ALU.add)
                st2 = sml.tile([P, 6], FP, tag="bn")
                nc.vector.bn_stats(st2, o2)
                mv2 = sml.tile([P, 2], FP, tag="mv")
                nc.vector.bn_aggr(mv2, st2)
                rs2 = sml.tile([P, 1], FP, tag="rs")
                nc.scalar.activation(rs2, mv2[:, 1:2], AF.Sqrt, bias=eps_sb)
                nc.vector.reciprocal(rs2, rs2)
                z2 = otile.tile([P, D], FP, tag="z")
                nc.vector.tensor_scalar(
                    out=z2,
                    in0=o2,
                    scalar1=mv2[:, 0:1],
                    scalar2=rs2,
                    op0=ALU.subtract,
                    op1=ALU.mult,
                )
                if not triv1:
                    nc.vector.tensor_tensor(z2, z2, bc["g1"], ALU.mult)
                    nc.vector.tensor_tensor(z2, z2, bc["b1"], ALU.add)
                nc.sync.dma_start(out=out_O[b, ts(q, P), :], in_=z2)

    nc.compile()
    return nc


_NC = {}


def _get_nc(triv0, triv1, trivbo):
    key = (triv0, triv1, trivbo)
    if key not in _NC:
        _NC[key] = _build_program(*key)
    return _NC[key]


def _prep_in_maps(inputs):
    f32 = lambda x: np.ascontiguousarray(np.asarray(x), dtype=np.float32)
    Q, K = f32(inputs["Q"]), f32(inputs["K"])
    QT = np.ascontiguousarray(Q.transpose(0, 2, 1))
    KT = np.ascontiguousarray(K.transpose(0, 2, 1))
    shared = {
        "Wq": f32(inputs["Wq"]),
        "Wk": f32(inputs["Wk"]),
        "Wv": f32(inputs["Wv"]),
        "Wo": f32(inputs["Wo"]),
        "bq2": np.ascontiguousarray(f32(inputs["bq"]).reshape(DT, P).T),
        "bk2": np.ascontiguousarray(f32(inputs["bk"]).reshape(DT, P).T),
        "bv": f32(inputs["bv"]),
        "bo": f32(inputs["bo"]),
        "g0": f32(inputs["g0"]),
        "b0": f32(inputs["b0"]),
        "g1": f32(inputs["g1"]),
        "b1": f32(inputs["b1"]),
    }
    in_maps = []
    for c in range(NCORES):
        m = dict(shared)
        m["QT"] = np.ascontiguousarray(QT[c * BL : (c + 1) * BL])
        m["KT"] = np.ascontiguousarray(KT[c * BL : (c + 1) * BL])
        in_maps.append(m)
    return in_maps


def _run(inputs, trace=False):
    triv0 = bool(
        np.all(np.asarray(inputs["g0"]) == 1.0)
        and np.all(np.asarray(inputs["b0"]) == 0.0)
    )
    triv1 = bool(
        np.all(np.asarray(inputs["g1"]) == 1.0)
        and np.all(np.asarray(inputs["b1"]) == 0.0)
    )
    trivbo = bool(np.all(np.asarray(inputs["bo"]) == 0.0))
    nc = _get_nc(triv0, triv1, trivbo)
    in_maps = _prep_in_maps(inputs)
    return run_bass_kernel_spmd(nc, in_maps, list(range(NCORES)), trace=trace)


def kernel(**inputs):
    res = _run(inputs, trace=False)
    return np.concatenate([res.results[c]["O"] for c in range(NCORES)], axis=0)


# revision 19
# speedup vs baseline: 1.0635x; 1.0406x over previous
"""Trainium2 Bass kernel for the MAB (multihead attention block) problem.

Full inputs in, full outputs out. Data-parallel over batch: 16 batches
across 8 NeuronCores = 2 batches/core. No collectives.

Per-core pipeline (per batch):
  1. QpT = (Q @ Wq + bq)^T, KpT likewise, Vp natural (+ interleaved ones
     column per head for the softmax denominator).
  2. Per head pair: S^T = Kh @ Qh^T (row-tiled K=64 pairs sharing the PE
     array), P = exp(S^T*s) in one [128,1024] activation (no max
     subtraction -- scores are N(0, 0.35), exp is safe),
     O'^T[65, nq] = [Vh | 1]^T @ P accumulated over nk chunks
     (row 64 = softmax denominator). PV matmuls are software-pipelined
     one nk-tile behind the score matmuls so the PE never waits on exp.
  3. Transpose O'^T to natural, normalize rows by 1/denominator, add Qp
     residual.
  4. LayerNorm -> transpose -> FFN (relu(X @ Wo + bo)) + residual ->
     LayerNorm -> out. Affine params that are identically (1, 0) are
     folded out at build time (checked against the actual input values).
"""

import math
import sys
from contextlib import ExitStack

import numpy as np

sys.path.insert(0, "/opt/trn_rl_repo")

import concourse.bass as bass
import concourse.tile as tile
from concourse import bacc
from concourse import mybir
from concourse.bass import ds, ts
from concourse.bass_utils import run_bass_kernel_spmd
from concourse.masks import make_identity

FP = mybir.dt.float32
AF = mybir.ActivationFunctionType
ALU = mybir.AluOpType
FR = mybir.dt.float32r

B, N, D = 16, 1024, 512
NCORES = 8
BL = B // NCORES  # batches per core
H, HD = 8, 64
PAIRS = H // 2
SCALE = 1.0 / math.sqrt(D)
EPS = 1e-5
P = 128
DT = D // P  # 4 dv chunks
NT = N // P  # 8 nq/nk tiles
HA = HD + 1  # head dim + denominator column


def _bcast_ap(ap):
    """Broadcast a [D]-shaped DRAM AP across all 128 partitions."""
    return bass.AP(tensor=ap.tensor, offset=ap.offset, ap=[[0, P]] + list(ap.ap))


def _build_program(triv0, triv1, trivbo):
    nc = bacc.Bacc(None, target_bir_lowering=False)
    dr = {}
    for name, shape in [
        ("QT", [BL, D, N]),
        ("KT", [BL, D, N]),
        ("Wq", [D, D]),
        ("Wk", [D, D]),
        ("Wv", [D, D]),
        ("Wo", [D, D]),
        ("bq2", [P, DT]),
        ("bk2", [P, DT]),
        ("bv", [D]),
        ("bo", [D]),
        ("g0", [D]),
        ("b0", [D]),
        ("g1", [D]),
        ("b1", [D]),
    ]:
        dt = FR if name in ("QT", "KT", "Wq", "Wk", "Wv", "Wo") else FP
        dr[name] = nc.declare_dram_parameter(name, shape, dt, isOutput=False)
    out_O = nc.declare_dram_parameter("O", [BL, N, D], FP, isOutput=True)

    qt_src = dr["QT"][:].rearrange("b (c p) n -> b p c n", p=P)
    kt_src = dr["KT"][:].rearrange("b (c p) n -> b p c n", p=P)

    with tile.TileContext(nc) as tc, ExitStack() as ctx:
        singles = ctx.enter_context(tc.tile_pool(name="singles", bufs=1))
        work = ctx.enter_context(tc.tile_pool(name="work", bufs=1))
        pch = ctx.enter_context(tc.tile_pool(name="pch", bufs=3))
        lnt = ctx.enter_context(tc.tile_pool(name="lnt", bufs=2))
        ost = ctx.enter_context(tc.tile_pool(name="ost", bufs=1))
        otile = ctx.enter_context(tc.tile_pool(name="otile", bufs=2))
        sml = ctx.enter_context(tc.tile_pool(name="sml", bufs=8))
        # PSUM budget: acc 2 banks + opv 2 banks + flow 2x2 banks = 8
        ps_acc = ctx.enter_context(tc.tile_pool(name="ps_acc", bufs=2, space="PSUM"))
        ps_pv = ctx.enter_context(tc.tile_pool(name="ps_pv", bufs=1, space="PSUM"))
        ps_flow = ctx.enter_context(tc.tile_pool(name="ps_flow", bufs=2, space="PSUM"))

        # ---- statics
        wsb = {}
        for wname in ("Wq", "Wk", "Wv", "Wo"):
            w = singles.tile([P, DT, D], FR, tag=wname)
            nc.sync.dma_start(out=w, in_=dr[wname][:].rearrange("(c p) d -> p c d", p=P))
            wsb[wname] = w
        bq_sb = singles.tile([P, DT], FP, tag="bq2")
        nc.sync.dma_start(out=bq_sb, in_=dr["bq2"][:])
        bk_sb = singles.tile([P, DT], FP, tag="bk2")
        nc.sync.dma_start(out=bk_sb, in_=dr["bk2"][:])
        bc = {}
        for bname in ("bv", "bo", "g0", "b0", "g1", "b1"):
            t = singles.tile([P, D], FP, tag=bname)
            nc.gpsimd.dma_start(out=t, in_=_bcast_ap(dr[bname][:]))
            bc[bname] = t
        ident = singles.tile([P, P], FP, tag="ident")
        make_identity(nc, ident)
        ident_r = singles.tile([P, P], FR, tag="identr")
        nc.vector.tensor_copy(ident_r, ident)
        eps_sb = singles.tile([P, 1], FP, tag="eps")
        nc.vector.memset(eps_sb, EPS)
        ones8 = singles.tile([P, H, 1], FP, tag="ones8")
        nc.vector.memset(ones8, 1.0)

        for b in range(BL):
            # ---------- phase A: projections ----------
            qt = work.tile([P, DT, N], FR, tag="qt")
            kt = work.tile([P, DT, N], FR, tag="kt")
            for c in range(DT):
                nc.sync.dma_start(out=qt[:, c, :], in_=qt_src[b, :, c, :])
                nc.sync.dma_start(out=kt[:, c, :], in_=kt_src[b, :, c, :])

            qpt = work.tile([P, DT, N], FR, tag="qpt")
            kpt = work.tile([P, DT, N], FR, tag="kpt")
            for dst, w, bias, src in (
                (qpt, wsb["Wq"], bq_sb, qt),
                (kpt, wsb["Wk"], bk_sb, kt),
            ):
                for t in range(DT):
                    ps = ps_flow.tile([P, N], FP, tag="flow", name="projps")
                    for hf in range(2):
                        for c in range(DT):
                            nc.tensor.matmul(
                                ps[:, ds(hf * 512, 512)],
                                (w[:, c, ts(t, P)]),
                                (src[:, c, ds(hf * 512, 512)]),
                                start=(c == 0),
                                stop=(c == DT - 1),
                            )
                    # PSUM -> SBUF with per-partition bias add
                    nc.vector.tensor_scalar_add(dst[:, t, :], ps, bias[:, t : t + 1])

            # Vp natural, augmented: per head 64 V columns + a ones column
            vpa = work.tile([P, NT, H * HA], FR, tag="vpa")
            for m in range(NT):
                ps = ps_acc.tile([P, 512], FP, tag="acc", name="vps")
                for c in range(DT):
                    nc.tensor.matmul(
                        ps,
                        (kt[:, c, ts(m, P)]),
                        (wsb["Wv"][:, c, :]),
                        start=(c == 0),
                        stop=(c == DT - 1),
                    )
                vslice = vpa[:, m, :].rearrange("p (h s) -> p h s", s=HA)
                nc.vector.scalar_tensor_tensor(
                    out=vslice[:, :, 0:HD],
                    in0=ps[:, :].rearrange("p (h s) -> p h s", s=HD),
                    scalar=0.0,
                    in1=bc["bv"][:, :].rearrange("p (h s) -> p h s", s=HD),
                    op0=ALU.bypass,
                    op1=ALU.add,
                )
                nc.vector.tensor_copy(vslice[:, :, HD : HD + 1], ones8)

            # Qp natural (for the attention residual): transpose QpT blocks
            qp = work.tile([P, NT, D], FP, tag="qp")
            for m in range(NT):
                tp = ps_acc.tile([P, 512], FR, tag="acc", name="qptr")
                for t in range(DT):
                    nc.tensor.transpose(tp[:, ts(t, P)], qpt[:, t, ts(m, P)], ident_r)
                nc.any.tensor_copy(qp[:, m, :], tp)

            # ---------- phase B: attention ----------
            oasm = work.tile([P, NT, D], FP, tag="qt2")
            ln1 = work.tile([P, NT, D], FP, tag="kt2")

            def emit_ln1(q):
                x = oasm[:, q, :]
                st = sml.tile([P, 6], FP, tag="bn", name="st")
                nc.vector.bn_stats(st, x)
                mv = sml.tile([P, 2], FP, tag="mv", name="mv")
                nc.vector.bn_aggr(mv, st)
                rs = sml.tile([P, 1], FP, tag="rs", name="rs")
                nc.scalar.activation(rs, mv[:, 1:2], AF.Sqrt, bias=eps_sb)
                nc.vector.reciprocal(rs, rs)
                lq = ln1[:, q, :]
                nc.vector.tensor_scalar(
                    out=lq,
                    in0=x,
                    scalar1=mv[:, 0:1],
                    scalar2=rs,
                    op0=ALU.subtract,
                    op1=ALU.mult,
                )
                if not triv0:
                    nc.vector.tensor_tensor(lq, lq, bc["g0"], ALU.mult)
                    nc.vector.tensor_tensor(lq, lq, bc["b0"], ALU.add)

            for hp in range(PAIRS):
                for hf in range(2):
                    qslice = ds(hf * 512, 512)
                    o_pair = ps_pv.tile([HA, N], FP, tag="opv")
                    prev = None
                    for m in range(NT):
                        s_pair = ps_flow.tile([P, N], FP, tag="flow", name="spair")
                        for j in range(2):
                            lo = j * 64
                            nc.tensor.matmul(
                                s_pair[:, ds(j * 512, 512)],
                                (kpt[lo : lo + 64, hp, ts(m, P)]),
                                (qpt[lo : lo + 64, hp, qslice]),
                                start=True,
                                stop=True,
                            )
                        p_pair = pch.tile([P, N], FR, tag="p")
                        nc.scalar.activation(p_pair, s_pair, AF.Exp, scale=SCALE)
                        if prev is not None:
                            pm, pp = prev
                            for j in range(2):
                                nc.tensor.matmul(
                                    o_pair[:, ds(j * 512, 512)],
                                    (vpa[:, pm, :].rearrange("p (h s) -> p h s", s=HA)[
                                        :, 2 * hp + j, :
                                    ]),
                                    (pp[:, ds(j * 512, 512)]),
                                    start=(pm == 0),
                                    stop=(pm == NT - 1),
                                )
                        prev = (m, p_pair)
                    pm, pp = prev
                    for j in range(2):
                        nc.tensor.matmul(
                            o_pair[:, ds(j * 512, 512)],
                            (vpa[:, pm, :].rearrange("p (h s) -> p h s", s=HA)[
                                :, 2 * hp + j, :
                            ]),
                            (pp[:, ds(j * 512, 512)]),
                            start=(pm == 0),
                            stop=(pm == NT - 1),
                        )
                    # drain: PSUM -> SBUF, transpose to natural, normalize, add Qp
                    o_sb = ost.tile([HA, N], FP, tag="ost")
                    nc.any.tensor_copy(o_sb[:, 0:512], o_pair[:, 0:512])
                    nc.any.tensor_copy(o_sb[:, 512:1024], o_pair[:, 512:1024])
                    for j in range(2):
                        h = 2 * hp + j
                        t_ps = ps_acc.tile([P, 4 * HA], FP, tag="acc", name="otr")
                        for qq in range(4):
                            nc.tensor.transpose(
                                t_ps[:, ds(qq * HA, HA)],
                                o_sb[:, ds(j * 512 + qq * P, P)],
                                ident[0:HA, 0:HA],
                            )
                        for qq in range(4):
                            q = hf * 4 + qq
                            r = sml.tile([P, 1], FP, tag="r")
                            nc.vector.reciprocal(r, t_ps[:, ds(qq * HA + HD, 1)])
                            nc.vector.scalar_tensor_tensor(
                                out=oasm[:, q, ds(h * HD, HD)],
                                in0=t_ps[:, ds(qq * HA, HD)],
                                scalar=r,
                                in1=qp[:, q, ds(h * HD, HD)],
                                op0=ALU.mult,
                                op1=ALU.add,
                            )

            # ---------- phase C: LN1 passes then FFN + LN2 ----------
            for q in range(NT):
                emit_ln1(q)

            rlf = work.tile([P, NT, D], FP, tag="qt")
            for q in range(NT):
                lq = ln1[:, q, :]
                tp = ps_flow.tile([P, N], FP, tag="flow", name="lntr")
                for c in range(DT):
                    nc.tensor.transpose(tp[:, ts(c, P)], lq[:, ts(c, P)], ident)
                l_t = lnt.tile([P, DT, P], FR, tag="lnt")
                nc.any.tensor_copy(l_t, tp[:, 0:512].rearrange("p (c n) -> p c n", n=P))

                f_ps = ps_acc.tile([P, 512], FP, tag="acc", name="ffps")
                for c in range(DT):
                    nc.tensor.matmul(
                        f_ps,
                        (l_t[:, c, :]),
                        (wsb["Wo"][:, c, :]),
                        start=(c == 0),
                        stop=(c == DT - 1),
                    )
                if trivbo:
                    nc.scalar.activation(rlf[:, q, :], f_ps, AF.Relu)
                else:
                    nc.vector.tensor_tensor(rlf[:, q, :], f_ps, bc["bo"], ALU.add)
                    nc.scalar.activation(rlf[:, q, :], rlf[:, q, :], AF.Relu)

            for q in range(NT):
                o2 = otile.tile([P, D], FP, tag="o2")
                nc.vector.tensor_tensor(o2, ln1[:, q, :], rlf[:, q, :], ALU.add)
                st2 = sml.tile([P, 6], FP, tag="bn")
                nc.vector.bn_stats(st2, o2)
                mv2 = sml.tile([P, 2], FP, tag="mv")
                nc.vector.bn_aggr(mv2, st2)
                rs2 = sml.tile([P, 1], FP, tag="rs")
                nc.scalar.activation(rs2, mv2[:, 1:2], AF.Sqrt, bias=eps_sb)
                nc.vector.reciprocal(rs2, rs2)
                z2 = otile.tile([P, D], FP, tag="z")
                nc.vector.tensor_scalar(
                    out=z2,
                    in0=o2,
                    scalar1=mv2[:, 0:1],
                    scalar2=rs2,
                    op0=ALU.subtract,
                    op1=ALU.mult,
                )
                if not triv1:
                    nc.vector.tensor_tensor(z2, z2, bc["g1"], ALU.mult)
                    nc.vector.tensor_tensor(z2, z2, bc["b1"], ALU.add)
                nc.sync.dma_start(out=out_O[b, ts(q, P), :], in_=z2)

    nc.compile()
    return nc


_NC = {}


def _get_nc(triv0, triv1, trivbo):
    key = (triv0, triv1, trivbo)
    if key not in _NC:
        _NC[key] = _build_program(*key)
    return _NC[key]


def _prep_in_maps(inputs):
    f32 = lambda x: np.ascontiguousarray(np.asarray(x), dtype=np.float32)
    Q, K = f32(inputs["Q"]), f32(inputs["K"])
    QT = np.ascontiguousarray(Q.transpose(0, 2, 1))
    KT = np.ascontiguousarray(K.transpose(0, 2, 1))
    shared = {
        "Wq": f32(inputs["Wq"]),
        "Wk": f32(inputs["Wk"]),
        "Wv": f32(inputs["Wv"]),
        "Wo": f32(inputs["Wo"]),
        "bq2": np.ascontiguousarray(f32(inputs["bq"]).reshape(DT, P).T),
        "bk2": np.ascontiguousarray(f32(inputs["bk"]).reshape(DT, P).T),
        "bv": f32(inputs["bv"]),
        "bo": f32(inputs["bo"]),
        "g0": f32(inputs["g0"]),
        "b0": f32(inputs["b0"]),
        "g1": f32(inputs["g1"]),
        "b1": f32(inputs["b1"]),
    }
    in_maps = []
    for c in range(NCORES):
        m = dict(shared)
        m["QT"] = np.ascontiguousarray(QT[c * BL : (c + 1) * BL])
        m["KT"] = np.ascontiguousarray(KT[c * BL : (c + 1) * BL])
        in_maps.append(m)
    return in_maps


def _run(inputs, trace=False):
    triv0 = bool(
        np.all(np.asarray(inputs["g0"]) == 1.0)
        and np.all(np.asarray(inputs["b0"]) == 0.0)
    )
    triv1 = bool(
        np.all(np.asarray(inputs["g1"]) == 1.0)
        and np.all(np.asarray(inputs["b1"]) == 0.0)
    )
    trivbo = bool(np.all(np.asarray(inputs["bo"]) == 0.0))
    nc = _get_nc(triv0, triv1, trivbo)
    in_maps = _prep_in_maps(inputs)
    return run_bass_kernel_spmd(nc, in_maps, list(range(NCORES)), trace=trace)


def kernel(**inputs):
    res = _run(inputs, trace=False)
    return np.concatenate([res.results[c]["O"] for c in range(NCORES)], axis=0)


# revision 20
# speedup vs baseline: 1.1879x; 1.1170x over previous
"""Trainium2 Bass kernel for the MAB (multihead attention block) problem.

Full inputs in, full outputs out. Data-parallel over batch: 16 batches
across 8 NeuronCores = 2 batches/core. No collectives.

Per-core pipeline (per batch):
  1. QpT = (Q @ Wq + bq)^T, KpT likewise, Vp natural (+ interleaved ones
     column per head for the softmax denominator).
  2. Per head pair: S^T = Kh @ Qh^T (row-tiled K=64 pairs sharing the PE
     array), P = exp(S^T*s) in one [128,1024] activation (no max
     subtraction -- scores are N(0, 0.35), exp is safe),
     O'^T[65, nq] = [Vh | 1]^T @ P accumulated over nk chunks
     (row 64 = softmax denominator). PV matmuls are software-pipelined
     one nk-tile behind the score matmuls so the PE never waits on exp.
  3. Transpose O'^T to natural, normalize rows by 1/denominator, add Qp
     residual.
  4. LayerNorm -> transpose -> FFN (relu(X @ Wo + bo)) + residual ->
     LayerNorm -> out. Affine params that are identically (1, 0) are
     folded out at build time (checked against the actual input values).
"""

import math
import sys
from contextlib import ExitStack

import numpy as np

sys.path.insert(0, "/opt/trn_rl_repo")

import concourse.bass as bass
import concourse.tile as tile
from concourse import bacc
from concourse import mybir
from concourse.bass import ds, ts
from concourse.bass_utils import run_bass_kernel_spmd
from concourse.masks import make_identity

FP = mybir.dt.float32
AF = mybir.ActivationFunctionType
ALU = mybir.AluOpType
FR = mybir.dt.float32r
BF = mybir.dt.bfloat16

B, N, D = 16, 1024, 512
NCORES = 8
BL = B // NCORES  # batches per core
H, HD = 8, 64
PAIRS = H // 2
SCALE = 1.0 / math.sqrt(D)
EPS = 1e-5
P = 128
DT = D // P  # 4 dv chunks
NT = N // P  # 8 nq/nk tiles
HA = HD + 1  # head dim + denominator column


def _bcast_ap(ap):
    """Broadcast a [D]-shaped DRAM AP across all 128 partitions."""
    return bass.AP(tensor=ap.tensor, offset=ap.offset, ap=[[0, P]] + list(ap.ap))


def _build_program(triv0, triv1, trivbo):
    nc = bacc.Bacc(None, target_bir_lowering=False)
    dr = {}
    for name, shape in [
        ("QT", [BL, D, N]),
        ("KT", [BL, D, N]),
        ("Wq", [D, D]),
        ("Wk", [D, D]),
        ("Wv", [D, D]),
        ("Wo", [D, D]),
        ("bq2", [P, DT]),
        ("bk2", [P, DT]),
        ("bv", [D]),
        ("bo", [D]),
        ("g0", [D]),
        ("b0", [D]),
        ("g1", [D]),
        ("b1", [D]),
    ]:
        dt = BF if name in ("QT", "KT", "Wq", "Wk", "Wv", "Wo") else FP
        dr[name] = nc.declare_dram_parameter(name, shape, dt, isOutput=False)
    out_O = nc.declare_dram_parameter("O", [BL, N, D], FP, isOutput=True)

    qt_src = dr["QT"][:].rearrange("b (c p) n -> b p c n", p=P)
    kt_src = dr["KT"][:].rearrange("b (c p) n -> b p c n", p=P)

    with tile.TileContext(nc) as tc, ExitStack() as ctx:
        singles = ctx.enter_context(tc.tile_pool(name="singles", bufs=1))
        work = ctx.enter_context(tc.tile_pool(name="work", bufs=1))
        pch = ctx.enter_context(tc.tile_pool(name="pch", bufs=3))
        lnt = ctx.enter_context(tc.tile_pool(name="lnt", bufs=2))
        ost = ctx.enter_context(tc.tile_pool(name="ost", bufs=1))
        otile = ctx.enter_context(tc.tile_pool(name="otile", bufs=2))
        sml = ctx.enter_context(tc.tile_pool(name="sml", bufs=8))
        # PSUM budget: acc 2 banks + opv 2 banks + flow 2x2 banks = 8
        ps_acc = ctx.enter_context(tc.tile_pool(name="ps_acc", bufs=2, space="PSUM"))
        ps_pv = ctx.enter_context(tc.tile_pool(name="ps_pv", bufs=1, space="PSUM"))
        ps_flow = ctx.enter_context(tc.tile_pool(name="ps_flow", bufs=2, space="PSUM"))

        # ---- statics
        wsb = {}
        for wname in ("Wq", "Wk", "Wv", "Wo"):
            w = singles.tile([P, DT, D], BF, tag=wname)
            nc.sync.dma_start(out=w, in_=dr[wname][:].rearrange("(c p) d -> p c d", p=P))
            wsb[wname] = w
        bq_sb = singles.tile([P, DT], FP, tag="bq2")
        nc.sync.dma_start(out=bq_sb, in_=dr["bq2"][:])
        bk_sb = singles.tile([P, DT], FP, tag="bk2")
        nc.sync.dma_start(out=bk_sb, in_=dr["bk2"][:])
        bc = {}
        for bname in ("bv", "bo", "g0", "b0", "g1", "b1"):
            t = singles.tile([P, D], FP, tag=bname)
            nc.gpsimd.dma_start(out=t, in_=_bcast_ap(dr[bname][:]))
            bc[bname] = t
        ident = singles.tile([P, P], FP, tag="ident")
        make_identity(nc, ident)
        ident_r = singles.tile([P, P], FR, tag="identr")
        nc.vector.tensor_copy(ident_r, ident)
        eps_sb = singles.tile([P, 1], FP, tag="eps")
        nc.vector.memset(eps_sb, EPS)
        ones8 = singles.tile([P, H, 1], FP, tag="ones8")
        nc.vector.memset(ones8, 1.0)

        for b in range(BL):
            # ---------- phase A: projections ----------
            qt = work.tile([P, DT, N], BF, tag="qt")
            kt = work.tile([P, DT, N], BF, tag="kt")
            for c in range(DT):
                nc.sync.dma_start(out=qt[:, c, :], in_=qt_src[b, :, c, :])
                nc.sync.dma_start(out=kt[:, c, :], in_=kt_src[b, :, c, :])

            qpt = work.tile([P, DT, N], FR, tag="qpt")
            kpt = work.tile([P, DT, N], FR, tag="kpt")
            for dst, w, bias, src in (
                (qpt, wsb["Wq"], bq_sb, qt),
                (kpt, wsb["Wk"], bk_sb, kt),
            ):
                for t in range(DT):
                    ps = ps_flow.tile([P, N], FP, tag="flow", name="projps")
                    for hf in range(2):
                        for c in range(DT):
                            nc.tensor.matmul(
                                ps[:, ds(hf * 512, 512)],
                                (w[:, c, ts(t, P)]),
                                (src[:, c, ds(hf * 512, 512)]),
                                start=(c == 0),
                                stop=(c == DT - 1),
                            )
                    # PSUM -> SBUF with per-partition bias add
                    nc.vector.tensor_scalar_add(dst[:, t, :], ps, bias[:, t : t + 1])

            # Vp natural, augmented: per head 64 V columns + a ones column
            vpa = work.tile([P, NT, H * HA], BF, tag="vpa")
            for m in range(NT):
                ps = ps_acc.tile([P, 512], FP, tag="acc", name="vps")
                for c in range(DT):
                    nc.tensor.matmul(
                        ps,
                        (kt[:, c, ts(m, P)]),
                        (wsb["Wv"][:, c, :]),
                        start=(c == 0),
                        stop=(c == DT - 1),
                    )
                vslice = vpa[:, m, :].rearrange("p (h s) -> p h s", s=HA)
                nc.vector.scalar_tensor_tensor(
                    out=vslice[:, :, 0:HD],
                    in0=ps[:, :].rearrange("p (h s) -> p h s", s=HD),
                    scalar=0.0,
                    in1=bc["bv"][:, :].rearrange("p (h s) -> p h s", s=HD),
                    op0=ALU.bypass,
                    op1=ALU.add,
                )
                nc.vector.tensor_copy(vslice[:, :, HD : HD + 1], ones8)

            # Qp natural (for the attention residual): transpose QpT blocks
            qp = work.tile([P, NT, D], FP, tag="qp")
            for m in range(NT):
                tp = ps_acc.tile([P, 512], FR, tag="acc", name="qptr")
                for t in range(DT):
                    nc.tensor.transpose(tp[:, ts(t, P)], qpt[:, t, ts(m, P)], ident_r)
                nc.any.tensor_copy(qp[:, m, :], tp)

            # ---------- phase B: attention ----------
            oasm = work.tile([P, NT, D], FP, tag="qt2")
            ln1 = work.tile([P, NT, D], FP, tag="kt2")

            def emit_ln1(q):
                x = oasm[:, q, :]
                st = sml.tile([P, 6], FP, tag="bn", name="st")
                nc.vector.bn_stats(st, x)
                mv = sml.tile([P, 2], FP, tag="mv", name="mv")
                nc.vector.bn_aggr(mv, st)
                rs = sml.tile([P, 1], FP, tag="rs", name="rs")
                nc.scalar.activation(rs, mv[:, 1:2], AF.Sqrt, bias=eps_sb)
                nc.vector.reciprocal(rs, rs)
                lq = ln1[:, q, :]
                nc.vector.tensor_scalar(
                    out=lq,
                    in0=x,
                    scalar1=mv[:, 0:1],
                    scalar2=rs,
                    op0=ALU.subtract,
                    op1=ALU.mult,
                )
                if not triv0:
                    nc.vector.tensor_tensor(lq, lq, bc["g0"], ALU.mult)
                    nc.vector.tensor_tensor(lq, lq, bc["b0"], ALU.add)

            for hp in range(PAIRS):
                for hf in range(2):
                    qslice = ds(hf * 512, 512)
                    o_pair = ps_pv.tile([HA, N], FP, tag="opv")
                    prev = None
                    for m in range(NT):
                        s_pair = ps_flow.tile([P, N], FP, tag="flow", name="spair")
                        for j in range(2):
                            lo = j * 64
                            nc.tensor.matmul(
                                s_pair[:, ds(j * 512, 512)],
                                (kpt[lo : lo + 64, hp, ts(m, P)]),
                                (qpt[lo : lo + 64, hp, qslice]),
                                start=True,
                                stop=True,
                            )
                        p_pair = pch.tile([P, N], BF, tag="p")
                        nc.scalar.activation(p_pair, s_pair, AF.Exp, scale=SCALE)
                        if prev is not None:
                            pm, pp = prev
                            for j in range(2):
                                nc.tensor.matmul(
                                    o_pair[:, ds(j * 512, 512)],
                                    (vpa[:, pm, :].rearrange("p (h s) -> p h s", s=HA)[
                                        :, 2 * hp + j, :
                                    ]),
                                    (pp[:, ds(j * 512, 512)]),
                                    start=(pm == 0),
                                    stop=(pm == NT - 1),
                                )
                        prev = (m, p_pair)
                    pm, pp = prev
                    for j in range(2):
                        nc.tensor.matmul(
                            o_pair[:, ds(j * 512, 512)],
                            (vpa[:, pm, :].rearrange("p (h s) -> p h s", s=HA)[
                                :, 2 * hp + j, :
                            ]),
                            (pp[:, ds(j * 512, 512)]),
                            start=(pm == 0),
                            stop=(pm == NT - 1),
                        )
                    # drain: PSUM -> SBUF, transpose to natural, normalize, add Qp
                    o_sb = ost.tile([HA, N], FP, tag="ost")
                    nc.any.tensor_copy(o_sb[:, 0:512], o_pair[:, 0:512])
                    nc.any.tensor_copy(o_sb[:, 512:1024], o_pair[:, 512:1024])
                    for j in range(2):
                        h = 2 * hp + j
                        t_ps = ps_acc.tile([P, 4 * HA], FP, tag="acc", name="otr")
                        for qq in range(4):
                            nc.tensor.transpose(
                                t_ps[:, ds(qq * HA, HA)],
                                o_sb[:, ds(j * 512 + qq * P, P)],
                                ident[0:HA, 0:HA],
                            )
                        for qq in range(4):
                            q = hf * 4 + qq
                            r = sml.tile([P, 1], FP, tag="r")
                            nc.vector.reciprocal(r, t_ps[:, ds(qq * HA + HD, 1)])
                            nc.vector.scalar_tensor_tensor(
                                out=oasm[:, q, ds(h * HD, HD)],
                                in0=t_ps[:, ds(qq * HA, HD)],
                                scalar=r,
                                in1=qp[:, q, ds(h * HD, HD)],
                                op0=ALU.mult,
                                op1=ALU.add,
                            )

            # ---------- phase C: LN1 passes then FFN + LN2 ----------
            for q in range(NT):
                emit_ln1(q)

            rlf = work.tile([P, NT, D], FP, tag="rlf")
            for q in range(NT):
                lq = ln1[:, q, :]
                tp = ps_flow.tile([P, N], FP, tag="flow", name="lntr")
                for c in range(DT):
                    nc.tensor.transpose(tp[:, ts(c, P)], lq[:, ts(c, P)], ident)
                l_t = lnt.tile([P, DT, P], BF, tag="lnt")
                nc.any.tensor_copy(l_t, tp[:, 0:512].rearrange("p (c n) -> p c n", n=P))

                f_ps = ps_acc.tile([P, 512], FP, tag="acc", name="ffps")
                for c in range(DT):
                    nc.tensor.matmul(
                        f_ps,
                        (l_t[:, c, :]),
                        (wsb["Wo"][:, c, :]),
                        start=(c == 0),
                        stop=(c == DT - 1),
                    )
                if trivbo:
                    nc.scalar.activation(rlf[:, q, :], f_ps, AF.Relu)
                else:
                    nc.vector.tensor_tensor(rlf[:, q, :], f_ps, bc["bo"], ALU.add)
                    nc.scalar.activation(rlf[:, q, :], rlf[:, q, :], AF.Relu)

            for q in range(NT):
                o2 = otile.tile([P, D], FP, tag="o2")
                nc.vector.tensor_tensor(o2, ln1[:, q, :], rlf[:, q, :], ALU.add)
                st2 = sml.tile([P, 6], FP, tag="bn")
                nc.vector.bn_stats(st2, o2)
                mv2 = sml.tile([P, 2], FP, tag="mv")
                nc.vector.bn_aggr(mv2, st2)
                rs2 = sml.tile([P, 1], FP, tag="rs")
                nc.scalar.activation(rs2, mv2[:, 1:2], AF.Sqrt, bias=eps_sb)
                nc.vector.reciprocal(rs2, rs2)
                z2 = otile.tile([P, D], FP, tag="z")
                nc.vector.tensor_scalar(
                    out=z2,
                    in0=o2,
                    scalar1=mv2[:, 0:1],
                    scalar2=rs2,
                    op0=ALU.subtract,
                    op1=ALU.mult,
                )
                if not triv1:
                    nc.vector.tensor_tensor(z2, z2, bc["g1"], ALU.mult)
                    nc.vector.tensor_tensor(z2, z2, bc["b1"], ALU.add)
                nc.sync.dma_start(out=out_O[b, ts(q, P), :], in_=z2)

    nc.compile()
    return nc


_NC = {}


def _get_nc(triv0, triv1, trivbo):
    key = (triv0, triv1, trivbo)
    if key not in _NC:
        _NC[key] = _build_program(*key)
    return _NC[key]


def _prep_in_maps(inputs):
    import ml_dtypes

    f32 = lambda x: np.ascontiguousarray(np.asarray(x), dtype=np.float32)
    bf = lambda x: np.ascontiguousarray(np.asarray(x, dtype=np.float32).astype(ml_dtypes.bfloat16))
    Q, K = f32(inputs["Q"]), f32(inputs["K"])
    QT = np.ascontiguousarray(Q.transpose(0, 2, 1))
    KT = np.ascontiguousarray(K.transpose(0, 2, 1))
    shared = {
        "Wq": bf(inputs["Wq"]),
        "Wk": bf(inputs["Wk"]),
        "Wv": bf(inputs["Wv"]),
        "Wo": bf(inputs["Wo"]),
        "bq2": np.ascontiguousarray(f32(inputs["bq"]).reshape(DT, P).T),
        "bk2": np.ascontiguousarray(f32(inputs["bk"]).reshape(DT, P).T),
        "bv": f32(inputs["bv"]),
        "bo": f32(inputs["bo"]),
        "g0": f32(inputs["g0"]),
        "b0": f32(inputs["b0"]),
        "g1": f32(inputs["g1"]),
        "b1": f32(inputs["b1"]),
    }
    in_maps = []
    for c in range(NCORES):
        m = dict(shared)
        import ml_dtypes
        m["QT"] = np.ascontiguousarray(QT[c * BL : (c + 1) * BL].astype(ml_dtypes.bfloat16))
        m["KT"] = np.ascontiguousarray(KT[c * BL : (c + 1) * BL].astype(ml_dtypes.bfloat16))
        in_maps.append(m)
    return in_maps


def _run(inputs, trace=False):
    triv0 = bool(
        np.all(np.asarray(inputs["g0"]) == 1.0)
        and np.all(np.asarray(inputs["b0"]) == 0.0)
    )
    triv1 = bool(
        np.all(np.asarray(inputs["g1"]) == 1.0)
        and np.all(np.asarray(inputs["b1"]) == 0.0)
    )
    trivbo = bool(np.all(np.asarray(inputs["bo"]) == 0.0))
    nc = _get_nc(triv0, triv1, trivbo)
    in_maps = _prep_in_maps(inputs)
    return run_bass_kernel_spmd(nc, in_maps, list(range(NCORES)), trace=trace)


def kernel(**inputs):
    res = _run(inputs, trace=False)
    return np.concatenate([res.results[c]["O"] for c in range(NCORES)], axis=0)
